# revision 6
# baseline (speedup 1.0000x reference)
"""Trainium2 Bass kernel for nn_EquivariantMLP_68745246540041.

Structure of the reference network: the output Linear only has a path from
the l=0 (scalar) block, and the scalar block of each Gate layer depends only
on the scalar block of its input.  So the live computation is

    y1 = x[:, :64] @ (W0_0[:, :64] * norm)          # (N, 64)
    s1 = CST * silu(y1)
    y2 = s1 @ (W1_0[:, :64] * norm)                 # (N, 64)
    s2 = CST * silu(y2)
    out = s2 @ (W_out * norm)                       # (N, 128)
    result = segment_sum(out, batch_indices, 512)   # (512, 128)

Device strategy (8 NeuronCores):
  - Segments (batch ids) are sharded across cores: core k owns segments
    [64k, 64k+64).  Atoms are grouped by segment on the host and placed into
    L-slot padded bins (zero padding - zeros are fixed points of the whole
    pipeline, so padded slots contribute nothing to the sums).
  - On-chip layout is "transposed + h-folded": partition p = h*64 + m where
    m is the feature index and h in {0,1} picks half of the core's segments.
    Weights become 128x128 block-diagonal matrices so one matmul processes
    both halves with full PE-array contraction width.
  - Per L-wide chunk (= one segment per half): matmul -> Silu (ScalarE LUT)
    -> matmul -> Silu with accum_out, which fuses the per-segment reduction
    into the activation.  The final W_out matmul is applied after the
    segment reduction (64 -> 128 on 64 columns only).
  - The CST / 1/sqrt(64) constants are folded into the weights on the host.
"""

import numpy as np

import concourse.bass as bass
import concourse.tile as tile
from concourse import mybir
from concourse.bass_utils import run_bass_kernel_spmd

F32 = mybir.dt.float32
F32R = mybir.dt.float32r

N_CORES = 8
H = 64

def _split_waits(nc, maxw: int = 1):
    """walrus' codegen rejects instructions carrying more than `maxw`
    semaphore waits.  Hoist excess waits onto nop instructions inserted
    immediately before the offender on the same engine stream — the engine
    stalls on the nops first, so semantics are identical."""
    for fn in nc.m.functions:
        for bb in fn.blocks:
            insts = bb.instructions
            if not any(
                inst.sync_info is not None
                and inst.sync_info.on_wait
                and len(inst.sync_info.on_wait) > maxw
                for inst in insts
            ):
                continue
            new = []
            for inst in insts:
                si = inst.sync_info
                if si is not None and si.on_wait and len(si.on_wait) > maxw:
                    waits = list(si.on_wait)
                    extra, keep = waits[:-maxw], waits[-maxw:]
                    for i in range(0, len(extra), maxw):
                        nop = mybir.InstNoOp(
                            name=nc.get_next_instruction_name(),
                            engine=inst.engine,
                            sync_info=mybir.SyncInfo(
                                on_wait=extra[i : i + maxw], on_update=[]
                            ),
                            bass_nofuse=True,
                        )
                        new.append(nop)
                    inst.sync_info = mybir.SyncInfo(
                        on_wait=keep,
                        on_update=list(si.on_update) if si.on_update else [],
                    )
                new.append(inst)
            bb.instructions = new


def _cst() -> np.float32:
    # e3nn normalize2mom constant for SiLU, reproduced exactly as in the
    # reference (np.random.default_rng(0), 1e6 samples).
    z = np.random.default_rng(0).standard_normal(1_000_000)
    s = z / (1.0 + np.exp(-z))
    return np.float32(1.0 / np.sqrt(np.mean(s * s)))


def _block_diag2(a: np.ndarray) -> np.ndarray:
    k, m = a.shape
    out = np.zeros((2 * k, 2 * m), np.float32)
    out[:k, :m] = a
    out[k:, m:] = a
    return np.ascontiguousarray(out)


def _build_program(L: int, s2: int, use_f32r: bool, use_silu_acc: bool):
    """Build the SPMD Bass program for per-half segment bin width L and
    per-half free width s2 = (segs_per_core/2) * L."""
    n_chunks = s2 // L

    nc = bass.Bass("TRN2", target_bir_lowering=False, debug=False)
    xt_d = nc.dram_tensor("xt", [128, s2], F32, kind="ExternalInput").ap()
    wa_d = nc.dram_tensor("wa", [128, 128], F32, kind="ExternalInput").ap()
    wb_d = nc.dram_tensor("wb", [128, 128], F32, kind="ExternalInput").ap()
    wc0_d = nc.dram_tensor("wc0", [128, 128], F32, kind="ExternalInput").ap()
    wc1_d = nc.dram_tensor("wc1", [128, 128], F32, kind="ExternalInput").ap()
    outa_d = nc.dram_tensor("outa", [128, n_chunks], F32, kind="ExternalOutput").ap()
    outb_d = nc.dram_tensor("outb", [128, n_chunks], F32, kind="ExternalOutput").ap()

    silu = mybir.ActivationFunctionType.Silu

    def mmdt(ap):
        return ap.bitcast(F32R) if use_f32r else ap

    with tile.TileContext(nc) as tc:
        with (
            tc.tile_pool(name="w", bufs=1) as wpool,
            tc.tile_pool(name="xin", bufs=6) as xpool,
            tc.tile_pool(name="act", bufs=4) as spool,
            tc.tile_pool(name="ps", bufs=3, space="PSUM") as ppool,
            tc.tile_pool(name="pso", bufs=1, space="PSUM") as oppool,
            tc.tile_pool(name="res", bufs=1) as rpool,
        ):
            wa = wpool.tile([128, 128], F32, tag="wa")
            nc.sync.dma_start(wa[:], wa_d[:])
            wb = wpool.tile([128, 128], F32, tag="wb")
            nc.sync.dma_start(wb[:], wb_d[:])
            wc0 = wpool.tile([128, 128], F32, tag="wc0")
            nc.sync.dma_start(wc0[:], wc0_d[:])
            wc1 = wpool.tile([128, 128], F32, tag="wc1")
            nc.sync.dma_start(wc1[:], wc1_d[:])

            segcols = rpool.tile([128, n_chunks], F32, tag="segcols")

            for j in range(n_chunks):
                xin = xpool.tile([128, L], F32, tag="xin")
                nc.sync.dma_start(xin[:], xt_d[:, j * L : (j + 1) * L])

                y1 = ppool.tile([128, L], F32, tag="y1")
                nc.tensor.matmul(
                    y1[:], mmdt(wa[:]), mmdt(xin[:]), start=True, stop=True
                )
                s1 = spool.tile([128, L], F32, tag="s1")
                nc.scalar.activation(s1[:], y1[:], silu)

                y2 = ppool.tile([128, L], F32, tag="y2")
                nc.tensor.matmul(
                    y2[:], mmdt(wb[:]), mmdt(s1[:]), start=True, stop=True
                )
                s2t = spool.tile([128, L], F32, tag="s2")
                if use_silu_acc:
                    nc.scalar.activation(
                        s2t[:], y2[:], silu, accum_out=segcols[:, j : j + 1]
                    )
                else:
                    nc.scalar.activation(s2t[:], y2[:], silu)
                    nc.vector.tensor_reduce(
                        segcols[:, j : j + 1],
                        s2t[:],
                        axis=mybir.AxisListType.X,
                        op=mybir.AluOpType.add,
                    )

            o0 = oppool.tile([128, n_chunks], F32, tag="o0")
            nc.tensor.matmul(o0[:], wc0[:], segcols[:], start=True, stop=True)
            o1 = oppool.tile([128, n_chunks], F32, tag="o1")
            nc.tensor.matmul(o1[:], wc1[:], segcols[:], start=True, stop=True)

            oa = rpool.tile([128, n_chunks], F32, tag="oa")
            nc.vector.tensor_copy(oa[:], o0[:])
            ob = rpool.tile([128, n_chunks], F32, tag="ob")
            nc.vector.tensor_copy(ob[:], o1[:])
            nc.sync.dma_start(outa_d[:], oa[:])
            nc.sync.dma_start(outb_d[:], ob[:])

    _split_waits(nc)
    return nc


def _prepare(x, batch_indices, W0_0, W1_0, W_out, batch_size):
    """Host-side layout: shard segments across cores, bin atoms into padded
    per-segment slots, transpose + h-fold, fold constants into weights."""
    B = int(batch_size)
    N = x.shape[0]
    assert B % N_CORES == 0
    segs_per_core = B // N_CORES
    assert segs_per_core % 2 == 0
    half = segs_per_core // 2

    bi = np.asarray(batch_indices).astype(np.int64).ravel()
    assert bi.shape[0] == N

    sizes = np.bincount(bi, minlength=B)
    maxseg = int(sizes.max())
    L = max(256, -(-maxseg // 64) * 64)
    assert L <= 512, f"segment of size {maxseg} exceeds supported bin width"
    s2 = half * L

    order = np.argsort(bi, kind="stable")
    starts = np.zeros(B + 1, np.int64)
    starts[1:] = np.cumsum(sizes)
    bi_sorted = bi[order]
    ranks = np.arange(N, dtype=np.int64) - starts[bi_sorted]
    dest = bi_sorted * L + ranks

    x64 = np.ascontiguousarray(np.asarray(x, dtype=np.float32)[:, :H])
    Xp = np.zeros((B * L, H), np.float32)
    Xp[dest] = x64[order]
    # (core, h, s2, m) -> (core, h, m, s2) -> (core, 128, s2)
    xt_all = np.ascontiguousarray(
        Xp.reshape(N_CORES, 2, s2, H).transpose(0, 1, 3, 2)
    ).reshape(N_CORES, 128, s2)

    norm = np.float32(1.0 / np.sqrt(H))
    cst = _cst()
    A = (np.asarray(W0_0, np.float32)[:, :H] * norm).astype(np.float32)
    Bw = (np.asarray(W1_0, np.float32)[:, :H] * (norm * cst)).astype(np.float32)
    C = (np.asarray(W_out, np.float32) * (norm * cst)).astype(np.float32)
    bdA = _block_diag2(A)
    bdB = _block_diag2(Bw)
    bdC0 = _block_diag2(C[:, :H])
    bdC1 = _block_diag2(C[:, H:])

    in_maps = [
        {
            "xt": xt_all[k],
            "wa": bdA,
            "wb": bdB,
            "wc0": bdC0,
            "wc1": bdC1,
        }
        for k in range(N_CORES)
    ]
    return in_maps, L, s2, half, B


def _assemble(results, half, B):
    out = np.zeros((B, 2 * H), np.float32)
    for k in range(N_CORES):
        oa = results[k]["outa"]
        ob = results[k]["outb"]
        for h in range(2):
            rows = slice(2 * half * k + h * half, 2 * half * k + (h + 1) * half)
            out[rows, :H] = oa[h * H : (h + 1) * H, :].T
            out[rows, H:] = ob[h * H : (h + 1) * H, :].T
    return out


def run(
    inputs: dict,
    use_f32r: bool = True,
    use_silu_acc: bool = True,
    trace: bool = False,
    **run_kwargs,
):
    in_maps, L, s2, half, B = _prepare(
        inputs["x"],
        inputs["batch_indices"],
        inputs["W0_0"],
        inputs["W1_0"],
        inputs["W_out"],
        inputs["batch_size"],
    )
    nc = _build_program(L, s2, use_f32r, use_silu_acc)
    res = run_bass_kernel_spmd(
        nc, in_maps, core_ids=list(range(N_CORES)), trace=trace, **run_kwargs
    )
    out = _assemble(res.results, half, B)
    return out, res


def kernel(**inputs) -> np.ndarray:
    out, _ = run(inputs)
    return out
